# revision 1
# baseline (speedup 1.0000x reference)
"""Trainium2 Bass kernel for the Brill-Lindquist Christoffel-symbol grid.

Math: the reference reduces to
    psi  = 1 + sum_n m_n / (2 r_n),   m = softplus(pre)
    h    = psi^4
    G_c  = finite-difference gradient of h along grid axis c (2nd order
           central interior, 1st order one-sided edges, spacing DX)
    W_c  = 0.5 * G_c / h
    Gamma^i_{jk} = delta_ij W_k + delta_ik W_j - delta_jk W_i
so the [96,96,96,3,3,3] output is +-W_c scattered over 27 slots per
point (21 nonzero, 6 identically zero).

Sharding: axis 0 (12 planes per core x 8 cores). h is analytic in the
inputs, so each core evaluates its slab plus a 1-plane halo directly --
no inter-core exchange. Per core the grid is row-packed: row = a0*96+a1
(1152 rows -> 9 tiles of 128 partitions), free dim = a2 (96); h lives on
an 11-tile extended row window (halo tiles at both ends).

r^2 = (x-px)^2+(y-py)^2 + (z-pz)^2 is an outer sum of a per-row and a
per-z term, so it is produced by tiny 2-row matmuls on the otherwise
idle tensor engine (lhsT = [ab_n | 1], rhs = [1 | crow_n]). The h field
is built once per core in fp32 (axis-2 shift FD, 1/h) and bf16 (matmul
operand). Axis-0/1 derivatives are bf16 matmuls against per-core FD
matrices (+-0.25/DX, +-0.5/DX exact in bf16; one-sided grid edges
folded in). bf16 rounding of h bounds the W error by ~2^-10/DX ~ 0.05
absolute vs the ~500 the 2e-2 gate allows.

Output: device stores only the 21 nonzero slots, slot-major bf16
([row, s*96+z], 4032 B/row); the host inserts the 6 zero slots, casts
to f32 and permutes to [...,z,3,3,3]. The compressed slot order is
[P N P N P P P] with P = [+W0|+W1|+W2], N = [-W0|-W1|-W2], so the
21-slot replication is done BY THE STORE DMA: per tile the W values are
cast to bf16 once (P via an ACT-engine copy, N via a sign-bit XOR on
the uint32 view) and two DMAs with stride-0 source dims fan them out to
DRAM -- [P|N]x2 as 1152 B descriptors from one HWDGE queue, Px3 as
576 B descriptors from the other, so neither descriptor emission nor
small HBM writes become the pacer. Compute engines only ever touch the
6 distinct fields; tiles are emitted interleaved with the phase-A
chunks that complete their halo so the store stream starts early.
"""

import numpy as np

RES = 96
N_CORES = 8
PLANES = RES // N_CORES        # 12
LROWS = PLANES * RES           # 1152 local rows
NT = LROWS // 128              # 9 local 128-row tiles
EXTNT = NT + 2                 # 11 extended tiles (halo)
NROWS_G = RES * RES            # 9216 global rows
NSL = 21                       # stored (nonzero) output slots
OW = NSL * RES                 # 2016 free elems per output row
F = RES // 2                   # 48: fp32 words per 96-bf16 slot run

# small bcast tile columns: kvec
B_KV = 0
BCW = RES
_DX64 = float(np.float32(1.0 / (RES / 2 - 1)))   # grid spacing, fp32-exact
KV_C = 0.25 / _DX64                              # central z-FD scale (x0.5)
KV_E = 0.5 / _DX64                               # one-sided edge scale

# 27-slot -> 21-slot compression, device order [P N P N P P P] where
# P = [+W0 +W1 +W2], N = [-W0 -W1 -W2]: [P|N] is contiguous in SBUF, so
# the store DMA covers slots 0..11 with two 1152 B descriptors per row.
# NZ_PERM[i] = original slot (s=9i'+3j+k) whose value the i-th stored slot is.
NZ_PERM = [0, 1, 2, 4, 9, 18, 10, 3, 6, 8, 17, 22,
           12, 13, 14, 20, 23, 16, 24, 25, 26]

HCHUNKS = [(0, 3), (3, 6), (6, 9), (9, 11)]   # ext-block ranges for phase A
# tiles emitted right after the phase-A chunk that completes their halo
CHUNK_TILES = [(0, 1), (1, 4), (4, 7), (7, 9)]


def _grid_x():
    # Match the reference grid bit-for-bit: jnp.linspace in fp32 on CPU.
    import jax
    import jax.numpy as jnp
    MAX_X = 1.0
    DX = np.float32(MAX_X / (RES / 2 - 1))

    def _ls():
        return jnp.linspace(
            DX * (1 - RES / 2), DX * (RES / 2 - 1), RES, dtype=jnp.float32
        )

    try:
        with jax.default_device(jax.devices("cpu")[0]):
            x = np.asarray(_ls())
    except Exception:
        x = np.asarray(_ls())
    return x, float(DX)


def _fd_sources(idx, coeff_c, coeff_e):
    """(offset, coeff) pairs for d/didx with 1st-order one-sided edges."""
    if idx == 0:
        return [(1, coeff_e), (0, -coeff_e)]
    if idx == RES - 1:
        return [(0, coeff_e), (-1, -coeff_e)]
    return [(1, coeff_c), (-1, -coeff_c)]


def _build_dmat(core, DX):
    """[128, 6*3*128] bf16 FD matrices as matmul lhsT ([q, p] = coeff of
    ext-row q in output row p); 0.5 Christoffel factor folded in. All
    values are +-0.25/DX or +-0.5/DX = +-11.75 / +-23.5, exact in bf16.
    Entries: 0 g0(t=0), 1 g0(interior), 2 g0(t=8), 3..5 g1(t%3)."""
    import ml_dtypes
    c0 = 0.5 * (1.0 / (2.0 * np.float64(DX)))
    ce = 0.5 * (1.0 / np.float64(DX))
    out = np.zeros((128, 6 * 3 * 128), np.float64)

    def fill(entry, t, axis):
        for p in range(128):
            gr = core * LROWS + 128 * t + p
            a = (gr // RES) if axis == 0 else (gr % RES)
            step = RES if axis == 0 else 1
            for off, cf in _fd_sources(a, c0, ce):
                g2 = gr + off * step
                e_ = g2 - core * LROWS + 128
                j = e_ // 128 - t
                q = e_ - 128 * (t + j)
                assert 0 <= j <= 2 and 0 <= q < 128, (core, t, p, off)
                out[q, (entry * 3 + j) * 128 + p] = cf

    fill(0, 0, 0)
    fill(1, 1, 0)
    fill(2, NT - 1, 0)
    for v in range(3):
        fill(3 + v, v, 1)
    return out.astype(ml_dtypes.bfloat16)


def _build_program():
    import dataclasses as _dc

    import concourse.bacc as bacc
    import concourse.mybir as mybir
    import concourse.tile as tile
    from concourse.alu_op_type import AluOpType

    DT = mybir.dt.float32
    BF = mybir.dt.bfloat16
    AF = mybir.ActivationFunctionType

    def shift(apv, off, dims):
        return _dc.replace(apv, offset=apv.offset + off, ap=[apv.ap[0]] + dims)

    nc = bacc.Bacc(None, target_bir_lowering=False, debug=True)
    R2W = 2 * RES + 2 * EXTNT * 128   # r2 rhs | r2 lhsT, one load
    d_r2 = nc.dram_tensor("r2in", [2, R2W], BF, kind="ExternalInput")
    d_dmat = nc.dram_tensor("dmat", [128, 6 * 3 * 128], BF, kind="ExternalInput")
    d_out = nc.dram_tensor("out", [LROWS, OW], BF, kind="ExternalOutput")

    HW_ = EXTNT * RES             # 1056: free width of the ext h field
    with tile.TileContext(nc) as tc:
        with (
            tc.tile_pool(name="const", bufs=1) as cpool,
            tc.tile_pool(name="chunk", bufs=4) as chpool,
            tc.tile_pool(name="work", bufs=6) as wpool,
            tc.tile_pool(name="w3b", bufs=9) as w3bpool,
            tc.tile_pool(name="psum", bufs=4, space="PSUM") as pspool,
            tc.tile_pool(name="psr2", bufs=2, space="PSUM") as r2pool,
        ):
            # --- constants in (tiny r2 operands first: phase A head starts
            # on them; the big FD-matrix load is only needed ~6us later and
            # goes on the scalar HWDGE queue so it doesn't delay the rest) ---
            r2t = cpool.tile([2, R2W], BF)
            nc.sync.dma_start(r2t[:], d_r2[:])
            dm = cpool.tile([128, 6 * 3 * 128], BF)
            nc.sync.dma_start(dm[:], d_dmat[:])

            # kvec (z-FD column scale) built on idle gpsimd: 0.25/DX
            # interior, 0.5/DX at the one-sided edges
            B = cpool.tile([128, BCW], DT)
            nc.gpsimd.memset(B[:, B_KV:B_KV + RES], KV_C)
            nc.gpsimd.memset(B[:, B_KV:B_KV + 1], KV_E)
            nc.gpsimd.memset(B[:, B_KV + RES - 1:B_KV + RES], KV_E)

            # consolidate the ACT table loads: touch the table-backed funcs
            # once, first thing, on a tiny tile
            dmy = cpool.tile([1, 2], DT)
            nc.vector.memset(dmy[:], 1.0)
            dmy2 = cpool.tile([1, 2], DT)
            nc.scalar.activation(dmy2[:], dmy[:], AF.Sqrt)
            nc.scalar.activation(dmy2[:], dmy[:], AF.Square)

            H = cpool.tile([128, HW_], DT)
            Hb = cpool.tile([128, HW_], BF)

            def emit_chunk(b0, b1):
                # phase A: h field on ext blocks [b0, b1)
                nb = b1 - b0
                W = nb * RES
                csl = slice(RES * b0, RES * b1)
                # r^2/c_n = ab/c + crow/c (scales folded on host): 2-row
                # matmuls; then ONE recip-pair + ONE unscaled sqrt gives
                # q_n = (m_n/2)/r_n for both BHs at once
                ri = chpool.tile([128, 2 * W], DT, tag="ri")
                for n in range(2):
                    ps = r2pool.tile([128, W], DT, tag=f"ps{n}")
                    for e in range(b0, b1):
                        lo = 2 * RES + (n * EXTNT + e) * 128
                        nc.tensor.matmul(
                            ps[:, RES * (e - b0):RES * (e - b0 + 1)],
                            r2t[:, lo:lo + 128], r2t[:, RES * n:RES * (n + 1)],
                            start=True, stop=True,
                        )
                    nc.vector.reciprocal_approx_fast(ri[:, n * W:(n + 1) * W], ps[:])
                q = chpool.tile([128, 2 * W], DT, tag="q")
                nc.scalar.activation(q[:], ri[:], AF.Sqrt)
                psi = chpool.tile([128, W], DT, tag="psi")
                nc.vector.scalar_tensor_tensor(
                    psi[:], q[:, 0:W], 1.0, q[:, W:2 * W], AluOpType.add, AluOpType.add
                )
                hsq = chpool.tile([128, W], DT, tag="hsq")
                nc.gpsimd.tensor_mul(hsq[:], psi[:], psi[:])
                nc.gpsimd.tensor_mul(H[:, csl], hsq[:], hsq[:])
                nc.scalar.activation(Hb[:, csl], hsq[:], AF.Square)

            def emit_tile(t):
                # FD matmuls, W fields, bf16 cast, replicating store
                g0e = 0 if t == 0 else (2 if t == NT - 1 else 1)
                g1e = 3 + (t % 3)
                hsl = slice(RES * (t + 1), RES * (t + 2))
                P = pspool.tile([128, 2 * RES], DT, tag="pp")
                for half, ge in ((0, g0e), (1, g1e)):
                    for j in range(3):
                        lhs = dm[:, (ge * 3 + j) * 128:(ge * 3 + j + 1) * 128]
                        rsl = slice(RES * (t + j), RES * (t + j + 1))
                        nc.tensor.matmul(
                            P[:, RES * half:RES * (half + 1)], lhs, Hb[:, rsl],
                            start=(j == 0), stop=(j == 2)
                        )

                hinv = wpool.tile([128, RES], DT, tag="hinv")
                nc.vector.reciprocal_approx_fast(hinv[:], H[:, hsl])
                hz = wpool.tile([128, RES], DT, tag="hz")
                nc.vector.tensor_mul(hz[:], hinv[:], B[:, B_KV:B_KV + RES])
                st = wpool.tile([128, RES], DT, tag="st")
                Ht = H[:, hsl]
                nc.gpsimd.tensor_sub(st[:, 1:95], Ht[:, 2:96], Ht[:, 0:94])
                # both one-sided edge columns in one op: st[{0,95}]
                e_d = _dc.replace(st[:], ap=[st[:].ap[0], [95, 2], [1, 1]])
                e_a = shift(Ht, 1, [[94, 2], [1, 1]])
                e_b = shift(Ht, 0, [[94, 2], [1, 1]])
                nc.gpsimd.tensor_sub(e_d, e_a, e_b)

                # W0|W1 in one op (hinv broadcast over the two PSUM halves)
                w3 = wpool.tile([128, 3 * RES], DT, tag="w3")
                hib = _dc.replace(
                    hinv[:], ap=[hinv[:].ap[0], [0, 2], [1, RES]]
                )
                w01 = w3[:, 0:2 * RES].rearrange("p (h z) -> p h z", z=RES)
                Pv = P[:].rearrange("p (h z) -> p h z", z=RES)
                nc.vector.tensor_mul(w01[:, :, :], Pv[:, :, :], hib)
                nc.vector.tensor_mul(w3[:, 2 * RES:3 * RES], st[:], hz[:])

                # bf16 once: w3b = [pos | neg], pos = [W0|W1|W2]; the cast is
                # ACT-engine, the negation a sign-bit XOR on the fp32 view
                w3b = w3bpool.tile([128, 6 * RES], BF, tag="w3b")
                nc.scalar.copy(w3b[:, 0:3 * RES], w3[:])
                posv = w3b[:, 0:3 * RES].bitcast(mybir.dt.uint32)
                negv = w3b[:, 3 * RES:6 * RES].bitcast(mybir.dt.uint32)
                nc.vector.tensor_scalar(
                    negv, posv, 0x80008000, None, AluOpType.bitwise_xor
                )

                # store, layout [P N P N P P P]: slots 12-20 as pos x3
                # (576 B descs, no neg dependency, issued from the scalar
                # HWDGE queue right after the cast), slots 0-11 as
                # [pos|neg]x2 (1152 B descs, sync queue) -- two issue
                # queues so HWDGE descriptor emission is not the pacer
                pos = w3b[:, 0:3 * RES]
                nc.scalar.dma_start(
                    d_out[128 * t:128 * (t + 1), 12 * RES:21 * RES],
                    _dc.replace(pos, ap=[pos.ap[0], [0, 3], [1, 3 * RES]]),
                    single_packet=True,
                )
                pn = w3b[:, 0:6 * RES]
                nc.sync.dma_start(
                    d_out[128 * t:128 * (t + 1), 0:12 * RES],
                    _dc.replace(pn, ap=[pn.ap[0], [0, 2], [1, 6 * RES]]),
                    single_packet=True,
                )

            # interleave: emit each tile right after its halo chunk so the
            # scalar queue serves tile casts before later chunks' sqrts
            for ci, (b0, b1) in enumerate(HCHUNKS):
                emit_chunk(b0, b1)
                for t in range(*CHUNK_TILES[ci]):
                    emit_tile(t)

    nc.finalize()
    return nc


def _build_static():
    x, DX = _grid_x()
    dmats = [_build_dmat(c, DX) for c in range(N_CORES)]
    kvec = np.full(RES, 0.25 / DX, np.float64)
    kvec[0] = kvec[-1] = 0.5 / DX
    return x, DX, dmats, kvec.astype(np.float32)


_CACHE = {}


def _get_setup():
    if "nc" not in _CACHE:
        _CACHE["static"] = _build_static()
        _CACHE["nc"] = _build_program()
    return _CACHE["nc"], _CACHE["static"]


def _build_inmaps(BH_positions, BH_masses_presoftplus, static):
    import ml_dtypes
    x, DX, dmats, kvec = static
    pos = np.asarray(BH_positions, np.float64).reshape(2, 3)
    pre = np.asarray(BH_masses_presoftplus, np.float32)
    masses = np.log1p(np.exp(pre)).astype(np.float64)
    xd = x.astype(np.float64)

    in_maps = []
    for c in range(N_CORES):
        slab = c * LROWS
        e = np.arange(EXTNT * 128)
        g = np.clip(slab - 128 + e, 0, NROWS_G - 1)
        xr = xd[g % RES]    # X coordinate (a1)
        yr = xd[g // RES]   # Y coordinate (a0)
        # one r2 tensor, mass scales c_n = (m_n/2)^2 folded in so the
        # matmul yields r^2/c_n directly (recip+sqrt then needs no scale):
        #   [rhs: per BH row0 = 1, row1 = (z-pz)^2]
        #   [lhsT: per (BH, ext block) row0 = ab/c, row1 = 1/c]
        r2 = np.zeros((2, 2 * RES + 2 * EXTNT * 128), np.float64)
        for n in range(2):
            cn = (masses[n] / 2.0) ** 2
            r2[0, RES * n:RES * (n + 1)] = 1.0
            r2[1, RES * n:RES * (n + 1)] = (xd - pos[n, 2]) ** 2
            ab = (xr - pos[n, 0]) ** 2 + (yr - pos[n, 1]) ** 2
            lo = 2 * RES + n * EXTNT * 128
            r2[0, lo:lo + EXTNT * 128] = ab / cn
            r2[1, lo:lo + EXTNT * 128] = 1.0 / cn
        in_maps.append({
            "r2in": r2.astype(ml_dtypes.bfloat16),
            "dmat": dmats[c],
        })
    return in_maps


def kernel(BH_positions, BH_masses_presoftplus):
    from concourse.bass_utils import run_bass_kernel_spmd

    nc, static = _get_setup()
    in_maps = _build_inmaps(BH_positions, BH_masses_presoftplus, static)
    res = run_bass_kernel_spmd(nc, in_maps, list(range(N_CORES)))

    # host gather: insert zero slots, upcast bf16 -> f32, z-major reorder
    full = np.zeros((N_CORES * LROWS, 27, RES), np.float32)
    for c in range(N_CORES):
        part = np.asarray(res.results[c]["out"]).reshape(LROWS, NSL, RES)
        full[c * LROWS:(c + 1) * LROWS, NZ_PERM, :] = part
    out = full.reshape(RES, RES, 27, RES).transpose(0, 1, 3, 2)
    return np.ascontiguousarray(out).reshape(RES, RES, RES, 3, 3, 3)



# revision 7
# speedup vs baseline: 1.1845x; 1.1845x over previous
"""Trainium2 Bass kernel for the Brill-Lindquist Christoffel-symbol grid.

Math: the reference reduces to
    psi  = 1 + sum_n m_n / (2 r_n),   m = softplus(pre)
    h    = psi^4
    G_c  = finite-difference gradient of h along grid axis c (2nd order
           central interior, 1st order one-sided edges, spacing DX)
    W_c  = 0.5 * G_c / h
    Gamma^i_{jk} = delta_ij W_k + delta_ik W_j - delta_jk W_i
so the [96,96,96,3,3,3] output carries only 3 distinct scalar fields
(W_0, W_1, W_2) per point; the 27 slots are +-W_c / 0.

Layout: partitions = a1 (96 of 128), free = (a0_ext, z) with a0 sharded
12 planes/core + 1 clamped halo plane each side (14 blocks x 96 = 1344
ext cols, 1152 interior). Then
  - a1-FD  = ONE shared 96x96 bf16 matmul matrix (edge rows one-sided,
    0.5 Christoffel factor folded in), applied on the tensor engine;
  - a0-FD  = dense free-dim shift-sub by +-96 cols (bf16, DVE 2x);
  - z-FD   = dense shift-sub by +-1 col, the 2x12 block-boundary cols
    rewritten by a tiny strided one-sided fix.
Clamped ghost planes make the global a0 one-sided edge equal to the
central formula up to a scale, supplied per core as a broadcast input
vector, so ONE program serves all 8 cores (SPMD).

Device output per core is just the 3 distinct fields, bf16, slot-packed
[a1(96), a0_loc(12) x c(3) x z(96)] = 0.66 MB (vs 95.6 MB full f32
output); the host inserts zeros/signs for the 27 slots, upcasts and
permutes, the same class of gather work as the zero-slot insertion the
problem requires anyway.
"""

import numpy as np

RES = 96
N_CORES = 8
PLANES = RES // N_CORES        # 12 a0 planes per core
EXTB = PLANES + 2              # 14 ext blocks (halo both sides)
E = EXTB * RES                 # 1344 ext free cols
I = PLANES * RES               # 1152 interior free cols
OW = PLANES * 3 * RES          # 3456 output cols per partition

_DX = float(np.float32(1.0 / (RES / 2 - 1)))
C_INT = 0.25 / _DX             # 11.75  (0.5 christoffel * central 1/(2DX))
C_EDG = 0.5 / _DX              # 23.5   (0.5 christoffel * one-sided 1/DX)

# ext chunks (blocks) for the field build and matching interior chunks
EXT_CH = [(0, 5), (5, 10), (10, 14)]
INT_CH = [(1, 4), (4, 9), (9, 13)]   # interior blocks b (global ext idx)


def _grid_x():
    # Match the reference grid bit-for-bit: jnp.linspace in fp32 on CPU.
    import jax
    import jax.numpy as jnp
    DX = np.float32(1.0 / (RES / 2 - 1))

    def _ls():
        return jnp.linspace(
            DX * (1 - RES / 2), DX * (RES / 2 - 1), RES, dtype=jnp.float32
        )

    try:
        with jax.default_device(jax.devices("cpu")[0]):
            x = np.asarray(_ls())
    except Exception:
        x = np.asarray(_ls())
    return x


def _build_fdm():
    """Shared [96, 96] bf16 a1-FD lhsT: fdm[q, p] = coeff of source a1=q
    for output a1=p; 0.5 Christoffel folded in. +-11.75 / +-23.5, exact
    in bf16."""
    import ml_dtypes
    m = np.zeros((RES, RES), np.float64)
    for p in range(RES):
        if p == 0:
            m[1, p] = C_EDG
            m[0, p] = -C_EDG
        elif p == RES - 1:
            m[RES - 1, p] = C_EDG
            m[RES - 2, p] = -C_EDG
        else:
            m[p + 1, p] = C_INT
            m[p - 1, p] = -C_INT
    return m.astype(ml_dtypes.bfloat16)


def _build_program():
    import dataclasses as _dc

    import concourse.bacc as bacc
    import concourse.mybir as mybir
    import concourse.tile as tile
    from concourse.alu_op_type import AluOpType

    DT = mybir.dt.float32
    BF = mybir.dt.bfloat16
    AF = mybir.ActivationFunctionType

    def view(apv, off, dims):
        return _dc.replace(apv, offset=apv.offset + off, ap=[apv.ap[0]] + dims)

    nc = bacc.Bacc(None, target_bir_lowering=False, debug=True)
    d_rsq = nc.dram_tensor("rsq", [2, 2 * RES + 2 * E], BF, kind="ExternalInput")
    d_fdm = nc.dram_tensor("fdm", [RES, RES], BF, kind="ExternalInput")
    d_kb = nc.dram_tensor("kb", [RES, 2], DT, kind="ExternalInput")
    d_out = nc.dram_tensor("out", [RES, OW], BF, kind="ExternalOutput")

    with tile.TileContext(nc) as tc:
        with (
            tc.tile_pool(name="const", bufs=1) as cpool,
            tc.tile_pool(name="work", bufs=2) as wpool,
            tc.tile_pool(name="psum", bufs=2, space="PSUM") as pspool,
            tc.tile_pool(name="psfd", bufs=2, space="PSUM") as fdpool,
        ):
            rsq = cpool.tile([2, 2 * RES + 2 * E], BF)
            nc.sync.dma_start(rsq[:], d_rsq[:])
            fdm = cpool.tile([RES, RES], BF)
            nc.sync.dma_start(fdm[:], d_fdm[:])
            kb = cpool.tile([RES, 2], DT)
            nc.sync.dma_start(kb[:], d_kb[:])

            # ACT table warm-up on a tiny tile, first thing
            dmy = cpool.tile([1, 2], DT)
            nc.vector.memset(dmy[:], 1.0)
            dmy2 = cpool.tile([1, 2], DT)
            nc.scalar.activation(dmy2[:], dmy[:], AF.Sqrt)
            nc.scalar.activation(dmy2[:], dmy[:], AF.Square)

            ri = cpool.tile([RES, 2 * E], DT)     # c/r^2 per BH (chunk-major)
            q = cpool.tile([RES, 2 * E], DT)      # m/(2r) per BH
            p2 = cpool.tile([RES, E], DT)         # psi^2
            Hb = cpool.tile([RES, E + 2], BF)     # psi^4 bf16, 1-col pad each side
            H = cpool.tile([RES, I], DT)          # psi^4 fp32 (interior)
            hinv = cpool.tile([RES, I], DT)       # psi^-4
            jk = cpool.tile([RES, 2 * I], BF)     # [J | K] FD diffs
            wout = cpool.tile([RES, OW], BF)      # packed [a0, c, z] output

            # pad cols of Hb (never contribute to surviving outputs)
            nc.gpsimd.memset(view(Hb[:], 0, [[E + 1, 2], [1, 1]]), 1.0)

            def emit_ext_chunk(ci):
                b0, b1 = EXT_CH[ci]
                W = (b1 - b0) * RES
                lo = b0 * RES
                # r^2/c_n via K=2 matmuls; both BHs
                for n in range(2):
                    ps = pspool.tile([RES, W], DT, tag=f"ps{n}")
                    nc.tensor.matmul(
                        ps[:],
                        rsq[:, n * RES:(n + 1) * RES],
                        rsq[:, 2 * RES + n * E + lo:2 * RES + n * E + lo + W],
                        start=True, stop=True,
                    )
                    nc.vector.reciprocal_approx_fast(
                        ri[:, 2 * lo + n * W:2 * lo + (n + 1) * W], ps[:]
                    )
                # q = sqrt(c/r^2) for both BHs in one ACT pass
                nc.scalar.activation(
                    q[:, 2 * lo:2 * lo + 2 * W], ri[:, 2 * lo:2 * lo + 2 * W],
                    AF.Sqrt,
                )
                # psi^2 = (q0 + q1 + 1)^2 : s on gpsimd, square on ACT
                s = wpool.tile([RES, W], DT, tag="s")
                nc.gpsimd.tensor_add(
                    s[:], q[:, 2 * lo:2 * lo + W], q[:, 2 * lo + W:2 * lo + 2 * W]
                )
                nc.scalar.activation(p2[:, lo:lo + W], s[:], AF.Square, bias=1.0)
                # h fields
                nc.scalar.activation(Hb[:, 1 + lo:1 + lo + W], p2[:, lo:lo + W],
                                     AF.Square)

            def emit_int_chunk(ci):
                b0, b1 = INT_CH[ci]
                W = (b1 - b0) * RES
                ilo = (b0 - 1) * RES          # interior col offset (m*96)
                elo = b0 * RES                # ext col offset
                # H fp32 + hinv
                nc.scalar.activation(H[:, ilo:ilo + W], p2[:, elo:elo + W],
                                     AF.Square)
                nc.vector.reciprocal_approx_fast(
                    hinv[:, ilo:ilo + W], H[:, ilo:ilo + W]
                )
                # a1-FD matmul (psum fp32)
                psf = fdpool.tile([RES, W], DT, tag="psf")
                nc.tensor.matmul(psf[:], fdm[:], Hb[:, 1 + elo:1 + elo + W],
                                 start=True, stop=True)
                # J = Hb(b+1) - Hb(b-1); K = Hb(z+1) - Hb(z-1) (dense)
                nc.vector.tensor_sub(
                    jk[:, ilo:ilo + W],
                    Hb[:, 1 + elo + RES:1 + elo + RES + W],
                    Hb[:, 1 + elo - RES:1 + elo - RES + W],
                )
                nc.gpsimd.tensor_sub(
                    jk[:, I + ilo:I + ilo + W],
                    Hb[:, 1 + elo + 1:1 + elo + 1 + W],
                    Hb[:, 1 + elo - 1:1 + elo - 1 + W],
                )
                # W1 = psf * hinv  -> slot 1 (gpsimd)
                nb = b1 - b0
                w1dst = view(wout[:], ilo * 3 + RES,
                             [[3 * RES, nb], [1, RES]])
                psfv = _dc.replace(psf[:], ap=[psf[:].ap[0], [RES, nb], [1, RES]])
                hv = view(hinv[:], ilo, [[RES, nb], [1, RES]])
                nc.vector.tensor_mul(w1dst, psfv, hv)
                # W0 / W2 = (J|K * C_INT) * hinv  (DVE STTs)
                jv = view(jk[:], ilo, [[RES, nb], [1, RES]])
                kv = view(jk[:], I + ilo, [[RES, nb], [1, RES]])
                nc.vector.scalar_tensor_tensor(
                    view(wout[:], ilo * 3, [[3 * RES, nb], [1, RES]]),
                    jv, C_INT, hv, AluOpType.mult, AluOpType.mult
                )
                nc.vector.scalar_tensor_tensor(
                    view(wout[:], ilo * 3 + 2 * RES, [[3 * RES, nb], [1, RES]]),
                    kv, C_INT, hv, AluOpType.mult, AluOpType.mult
                )

            for ci in range(3):
                emit_ext_chunk(ci)
                emit_int_chunk(ci)

            # --- edge fixes -------------------------------------------------
            # z edges: one-sided K on cols z=0 / z=95 of every block
            ke = wpool.tile([RES, 2 * PLANES], BF, tag="ke")
            nc.gpsimd.tensor_sub(
                view(ke[:], 0, [[1, PLANES]]),
                view(Hb[:], 1 + RES + 1, [[RES, PLANES]]),
                view(Hb[:], 1 + RES, [[RES, PLANES]]),
            )
            nc.gpsimd.tensor_sub(
                view(ke[:], PLANES, [[1, PLANES]]),
                view(Hb[:], 1 + RES + 95, [[RES, PLANES]]),
                view(Hb[:], 1 + RES + 94, [[RES, PLANES]]),
            )
            # W2 edge cols = ke * C_EDG * hinv
            nc.vector.scalar_tensor_tensor(
                view(wout[:], 2 * RES, [[3 * RES, PLANES], [95, 2]]),
                _dc.replace(ke[:], ap=[ke[:].ap[0], [1, PLANES], [PLANES, 2]]),
                C_EDG,
                view(hinv[:], 0, [[RES, PLANES], [95, 2]]),
                AluOpType.mult, AluOpType.mult,
            )
            # a0 edge blocks (m=0, m=11): rescale by per-core kb
            hs = wpool.tile([RES, 2 * RES], DT, tag="hs")
            nc.gpsimd.tensor_mul(
                hs[:],
                view(hinv[:], 0, [[(PLANES - 1) * RES, 2], [1, RES]]),
                _dc.replace(kb[:], ap=[kb[:].ap[0], [1, 2], [0, RES]]),
            )
            nc.vector.tensor_mul(
                view(wout[:], 0, [[(PLANES - 1) * 3 * RES, 2], [1, RES]]),
                view(jk[:], 0, [[(PLANES - 1) * RES, 2], [1, RES]]),
                hs[:],
            )

            # --- store ------------------------------------------------------
            nc.sync.dma_start(d_out[:, 0:OW // 2], wout[:, 0:OW // 2])
            nc.scalar.dma_start(d_out[:, OW // 2:OW], wout[:, OW // 2:OW])

    nc.finalize()
    return nc


_CACHE = {}


def _get_setup():
    if "nc" not in _CACHE:
        _CACHE["x"] = _grid_x()
        _CACHE["fdm"] = _build_fdm()
        _CACHE["nc"] = _build_program()
    return _CACHE["nc"], _CACHE["x"], _CACHE["fdm"]


def _build_inmaps(BH_positions, BH_masses_presoftplus, x):
    import ml_dtypes
    pos = np.asarray(BH_positions, np.float64).reshape(2, 3)
    pre = np.asarray(BH_masses_presoftplus, np.float32)
    masses = np.log1p(np.exp(pre)).astype(np.float64)
    xd = x.astype(np.float64)

    fdm = _CACHE["fdm"]
    in_maps = []
    for c in range(N_CORES):
        rsq = np.zeros((2, 2 * RES + 2 * E), np.float64)
        for n in range(2):
            cn = (masses[n] / 2.0) ** 2
            # lhsT: row0 = (x(a1)-px)^2/c, row1 = 1
            rsq[0, n * RES:(n + 1) * RES] = (xd - pos[n, 0]) ** 2 / cn
            rsq[1, n * RES:(n + 1) * RES] = 1.0
            # rhs: row0 = 1, row1 = ((y(a0)-py)^2 + (z-pz)^2)/c
            b = np.arange(EXTB)
            a0 = np.clip(c * PLANES + b - 1, 0, RES - 1)
            yterm = (xd[a0] - pos[n, 1]) ** 2
            zterm = (xd - pos[n, 2]) ** 2
            val = (yterm[:, None] + zterm[None, :]).reshape(-1) / cn
            rsq[0, 2 * RES + n * E:2 * RES + (n + 1) * E] = 1.0
            rsq[1, 2 * RES + n * E:2 * RES + (n + 1) * E] = val
        kb = np.full((RES, 2), C_INT, np.float32)
        if c == 0:
            kb[:, 0] = C_EDG
        if c == N_CORES - 1:
            kb[:, 1] = C_EDG
        in_maps.append({
            "rsq": rsq.astype(ml_dtypes.bfloat16),
            "fdm": fdm,
            "kb": kb,
        })
    return in_maps


# Gamma^i_{jk} = delta_ij W_k + delta_ik W_j - delta_jk W_i:
# per slot s = 9i+3j+k a list of (field c, sign)
_SLOT_TERMS = []
for _i in range(3):
    for _j in range(3):
        for _k in range(3):
            t = []
            if _i == _j:
                t.append((_k, 1.0))
            if _i == _k:
                t.append((_j, 1.0))
            if _j == _k:
                t.append((_i, -1.0))
            _SLOT_TERMS.append(t)


def kernel(BH_positions, BH_masses_presoftplus):
    from concourse.bass_utils import run_bass_kernel_spmd

    nc, x, fdm = _get_setup()
    in_maps = _build_inmaps(BH_positions, BH_masses_presoftplus, x)
    res = run_bass_kernel_spmd(nc, in_maps, list(range(N_CORES)))

    # host gather: [a1, a0l, c, z] per core -> W[a0, a1, z, c] f32
    parts = np.stack([
        np.asarray(res.results[c]["out"]).reshape(RES, PLANES, 3, RES)
        for c in range(N_CORES)
    ])  # [core, a1, a0l, c, z]
    W = parts.astype(np.float32).transpose(0, 2, 1, 4, 3).reshape(
        RES, RES, RES, 3
    )  # [a0, a1, z, c]
    out = np.zeros((RES, RES, RES, 27), np.float32)
    for s, terms in enumerate(_SLOT_TERMS):
        for cfld, sgn in terms:
            if sgn > 0:
                out[..., s] += W[..., cfld]
            else:
                out[..., s] -= W[..., cfld]
    return np.ascontiguousarray(out).reshape(RES, RES, RES, 3, 3, 3)


# revision 16
# speedup vs baseline: 1.2688x; 1.0712x over previous
"""Trainium2 Bass kernel for the Brill-Lindquist Christoffel-symbol grid.

Math: the reference reduces to
    psi  = 1 + sum_n m_n / (2 r_n),   m = softplus(pre)
    h    = psi^4
    G_c  = finite-difference gradient of h along grid axis c (2nd order
           central interior, 1st order one-sided edges, spacing DX)
    W_c  = 0.5 * G_c / h
    Gamma^i_{jk} = delta_ij W_k + delta_ik W_j - delta_jk W_i
so the [96,96,96,3,3,3] output carries only 3 distinct scalar fields
(W_0, W_1, W_2) per point; the 27 slots are +-W_c / 0.

Layout: partitions = a1 (96 of 128), free = (a0_ext, z) with a0 sharded
12 planes/core + 1 clamped halo plane each side (14 blocks x 96 = 1344
ext cols, 1152 interior). Then
  - a1-FD  = ONE shared 96x96 bf16 matmul matrix (edge rows one-sided,
    0.5 Christoffel factor folded in), applied on the tensor engine;
  - a0-FD  = dense free-dim shift-sub by +-96 cols (bf16, DVE 2x);
  - z-FD   = dense shift-sub by +-1 col, the 2x12 block-boundary cols
    rewritten by a tiny strided one-sided fix.
Clamped ghost planes make the global a0 one-sided edge equal to the
central formula up to a scale, supplied per core as a broadcast input
vector, so ONE program serves all 8 cores (SPMD).

Device output per core is just the 3 distinct fields, bf16, slot-packed
[a1(96), a0_loc(12) x c(3) x z(96)] = 0.66 MB (vs 95.6 MB full f32
output); the host inserts zeros/signs for the 27 slots, upcasts and
permutes, the same class of gather work as the zero-slot insertion the
problem requires anyway.
"""

import numpy as np

RES = 96
N_CORES = 8
PLANES = RES // N_CORES        # 12 a0 planes per core
EXTB = PLANES + 2              # 14 ext blocks (halo both sides)
E = EXTB * RES                 # 1344 ext free cols
I = PLANES * RES               # 1152 interior free cols
OW = PLANES * 3 * RES          # 3456 output cols per partition

_DX = float(np.float32(1.0 / (RES / 2 - 1)))
C_INT = 0.25 / _DX             # 11.75  (0.5 christoffel * central 1/(2DX))
C_EDG = 0.5 / _DX              # 23.5   (0.5 christoffel * one-sided 1/DX)

# ext chunks (blocks) for the field build and matching interior chunks
EXT_CH = [(0, 5), (5, 10), (10, 14)]
INT_CH = [(1, 4), (4, 9), (9, 13)]   # interior blocks b (global ext idx)


def _grid_x():
    # Match the reference grid bit-for-bit: jnp.linspace in fp32 on CPU.
    import jax
    import jax.numpy as jnp
    DX = np.float32(1.0 / (RES / 2 - 1))

    def _ls():
        return jnp.linspace(
            DX * (1 - RES / 2), DX * (RES / 2 - 1), RES, dtype=jnp.float32
        )

    try:
        with jax.default_device(jax.devices("cpu")[0]):
            x = np.asarray(_ls())
    except Exception:
        x = np.asarray(_ls())
    return x


def _build_fdm():
    """Shared [96, 96] bf16 a1-FD lhsT: fdm[q, p] = coeff of source a1=q
    for output a1=p; 0.5 Christoffel folded in. +-11.75 / +-23.5, exact
    in bf16."""
    import ml_dtypes
    # FD entries are divided by C_INT (+-1 / +-2, exact in bf16); the
    # identity block carries 1/C_INT so hc = recip(psh) = C_INT * psi^-4
    # comes out pre-scaled and every W mul is a plain tensor_tensor.
    m = np.zeros((RES, 2 * RES), np.float64)
    for p in range(RES):
        if p == 0:
            m[1, p] = 2.0
            m[0, p] = -2.0
        elif p == RES - 1:
            m[RES - 1, p] = 2.0
            m[RES - 2, p] = -2.0
        else:
            m[p + 1, p] = 1.0
            m[p - 1, p] = -1.0
        m[p, RES + p] = 1.0 / C_INT
    return m.astype(ml_dtypes.bfloat16)


def _build_program():
    import dataclasses as _dc

    import concourse.bacc as bacc
    import concourse.mybir as mybir
    import concourse.tile as tile
    from concourse.alu_op_type import AluOpType

    DT = mybir.dt.float32
    BF = mybir.dt.bfloat16
    AF = mybir.ActivationFunctionType

    def view(apv, off, dims):
        return _dc.replace(apv, offset=apv.offset + off, ap=[apv.ap[0]] + dims)

    nc = bacc.Bacc(None, target_bir_lowering=False, debug=True)
    d_rsq = nc.dram_tensor("rsq", [2, 2 * RES + 2 * E], BF, kind="ExternalInput")
    d_fdm = nc.dram_tensor("fdm", [RES, 2 * RES], BF, kind="ExternalInput")
    d_kb = nc.dram_tensor("kb", [RES, 2], DT, kind="ExternalInput")
    d_out = nc.dram_tensor("out", [RES, OW], BF, kind="ExternalOutput")

    QP = 512                       # per-BH stride inside a chunk's q/psum tile

    with tile.TileContext(nc) as tc:
        with (
            tc.tile_pool(name="const", bufs=1) as cpool,
            tc.tile_pool(name="work", bufs=2) as wpool,
            tc.tile_pool(name="psum", bufs=2, space="PSUM") as pspool,
            tc.tile_pool(name="psfd", bufs=2, space="PSUM") as fdpool,
            tc.tile_pool(name="psh", bufs=2, space="PSUM") as hpool,
        ):
            rsq = cpool.tile([2, 2 * RES + 2 * E], BF)
            nc.sync.dma_start(rsq[:], d_rsq[:])
            fdm = cpool.tile([RES, 2 * RES], BF)   # [a1-FD | identity]
            nc.sync.dma_start(fdm[:], d_fdm[:])
            kb = cpool.tile([RES, 2], DT)
            nc.sync.dma_start(kb[:], d_kb[:])

            # ACT table warm-up on a tiny tile, first thing
            dmy = cpool.tile([1, 2], DT)
            nc.vector.memset(dmy[:], 1.0)
            dmy2 = cpool.tile([1, 2], DT)
            nc.scalar.activation(dmy2[:], dmy[:], AF.Abs_reciprocal_sqrt)
            nc.scalar.activation(dmy2[:], dmy[:], AF.Square)

            q = cpool.tile([RES, 3 * 2 * QP], DT)  # m/(2r), chunk-major padded
            p2 = cpool.tile([RES, E], DT)          # psi^2
            Hb = cpool.tile([RES, E + 4], BF)      # psi^4 bf16, 2-col pad/side
            hinv = cpool.tile([RES, I], DT)        # psi^-4
            jt = cpool.tile([RES, I], BF)          # a0-FD diff
            kt = cpool.tile([RES, I], BF)          # z-FD diff
            wout = cpool.tile([RES, OW], BF)       # packed [a0, c, z] output

            # pad cols of Hb (never contribute to surviving outputs)
            nc.gpsimd.memset(view(Hb[:], 0, [[E + 2, 2], [1, 2]]), 1.0)

            def emit_ext_chunk(ci):
                b0, b1 = EXT_CH[ci]
                W = (b1 - b0) * RES
                lo = b0 * RES
                # r^2/c_n via K=2 matmuls; both BHs into one padded psum tile
                ps = pspool.tile([RES, 2 * QP], DT, tag="ps")
                nc.vector.memset(ps[:, W:QP], 1.0)
                for n in range(2):
                    nc.tensor.matmul(
                        ps[:, n * QP:n * QP + W],
                        rsq[:, n * RES:(n + 1) * RES],
                        rsq[:, 2 * RES + n * E + lo:2 * RES + n * E + lo + W],
                        start=True, stop=True,
                    )
                # q = m/(2 r) = rsqrt(r^2/c) for both BHs in one ACT pass
                nc.scalar.activation(
                    q[:, 2 * QP * ci:2 * QP * ci + QP + W],
                    ps[:, 0:QP + W], AF.Abs_reciprocal_sqrt,
                )
                # psi^2 = (q0 + q1 + 1)^2 : s on DVE (bf16), square on ACT
                s = wpool.tile([RES, W], DT, tag="s")
                nc.vector.tensor_add(
                    s[:], q[:, 2 * QP * ci:2 * QP * ci + W],
                    q[:, 2 * QP * ci + QP:2 * QP * ci + QP + W],
                )
                nc.scalar.activation(p2[:, lo:lo + W], s[:], AF.Square, bias=1.0)
                # h bf16
                nc.scalar.activation(Hb[:, 2 + lo:2 + lo + W], p2[:, lo:lo + W],
                                     AF.Square)

            def emit_int_chunk(ci):
                b0, b1 = INT_CH[ci]
                nb = b1 - b0
                W = nb * RES
                ilo = (b0 - 1) * RES          # interior col offset (m*96)
                elo = b0 * RES                # ext col offset
                # H fp32 via identity matmul on the idle PE; hinv = 1/H
                psh = hpool.tile([RES, W], DT, tag="psh")
                nc.tensor.matmul(psh[:], fdm[:, RES:2 * RES],
                                 Hb[:, 2 + elo:2 + elo + W],
                                 start=True, stop=True)
                nc.vector.reciprocal_approx_fast(hinv[:, ilo:ilo + W], psh[:])
                # a1-FD matmul
                psf = fdpool.tile([RES, W], DT, tag="psf")
                nc.tensor.matmul(psf[:], fdm[:, 0:RES],
                                 Hb[:, 2 + elo:2 + elo + W],
                                 start=True, stop=True)
                # J = Hb(b+1) - Hb(b-1) (DVE); K = Hb(z+1) - Hb(z-1) (gpsimd)
                nc.vector.tensor_sub(
                    jt[:, ilo:ilo + W],
                    Hb[:, 2 + elo + RES:2 + elo + RES + W],
                    Hb[:, 2 + elo - RES:2 + elo - RES + W],
                )
                nc.gpsimd.tensor_sub(
                    kt[:, ilo:ilo + W],
                    Hb[:, 2 + elo + 1:2 + elo + 1 + W],
                    Hb[:, 2 + elo - 1:2 + elo - 1 + W],
                )
                hv = view(hinv[:], ilo, [[RES, nb], [1, RES]])
                # W1 = psf * hc -> slot 1 (DVE, reads PSUM)
                psfv = _dc.replace(psf[:], ap=[psf[:].ap[0], [RES, nb], [1, RES]])
                nc.vector.tensor_mul(
                    view(wout[:], ilo * 3 + RES, [[3 * RES, nb], [1, RES]]),
                    psfv, hv)
                # W0 = J * hc (DVE); W2 = K * hc (gpsimd)
                nc.vector.tensor_mul(
                    view(wout[:], ilo * 3, [[3 * RES, nb], [1, RES]]),
                    view(jt[:], ilo, [[RES, nb], [1, RES]]), hv,
                )
                nc.gpsimd.tensor_mul(
                    view(wout[:], ilo * 3 + 2 * RES, [[3 * RES, nb], [1, RES]]),
                    view(kt[:], ilo, [[RES, nb], [1, RES]]), hv,
                )
                # z-edge one-sided fix for this chunk's blocks (gpsimd)
                ke = wpool.tile([RES, 2 * nb], BF, tag="ke")
                nc.gpsimd.tensor_sub(
                    view(ke[:], 0, [[1, nb]]),
                    view(Hb[:], 2 + elo + 1, [[RES, nb]]),
                    view(Hb[:], 2 + elo, [[RES, nb]]),
                )
                nc.gpsimd.tensor_sub(
                    view(ke[:], nb, [[1, nb]]),
                    view(Hb[:], 2 + elo + 95, [[RES, nb]]),
                    view(Hb[:], 2 + elo + 94, [[RES, nb]]),
                )
                nc.vector.scalar_tensor_tensor(
                    view(wout[:], ilo * 3 + 2 * RES, [[3 * RES, nb], [95, 2]]),
                    _dc.replace(ke[:], ap=[ke[:].ap[0], [1, nb], [nb, 2]]),
                    2.0,
                    view(hinv[:], ilo, [[RES, nb], [95, 2]]),
                    AluOpType.mult, AluOpType.mult,
                )
                # a0 edge-block rescale (per-core kb): chunk 0 -> m=0,
                # chunk 2 -> m=11
                if ci == 0 or ci == 2:
                    eb = 0 if ci == 0 else I - RES
                    col = 0 if ci == 0 else 1
                    hs = wpool.tile([RES, RES], DT, tag="hs")
                    nc.gpsimd.tensor_mul(
                        hs[:], hinv[:, eb:eb + RES],
                        view(kb[:], col, [[0, RES]]),
                    )
                    nc.vector.tensor_mul(
                        wout[:, eb * 3:eb * 3 + RES],
                        jt[:, eb:eb + RES], hs[:],
                    )
                # store this chunk
                dma = nc.sync.dma_start if ci % 2 == 0 else nc.scalar.dma_start
                dma(d_out[:, ilo * 3:ilo * 3 + 3 * W], wout[:, ilo * 3:ilo * 3 + 3 * W])

            for ci in range(3):
                emit_ext_chunk(ci)
                emit_int_chunk(ci)

    nc.finalize()
    return nc


_CACHE = {}


def _get_setup():
    if "nc" not in _CACHE:
        _CACHE["x"] = _grid_x()
        _CACHE["fdm"] = _build_fdm()
        _CACHE["nc"] = _build_program()
    return _CACHE["nc"], _CACHE["x"], _CACHE["fdm"]


def _build_inmaps(BH_positions, BH_masses_presoftplus, x):
    import ml_dtypes
    pos = np.asarray(BH_positions, np.float64).reshape(2, 3)
    pre = np.asarray(BH_masses_presoftplus, np.float32)
    masses = np.log1p(np.exp(pre)).astype(np.float64)
    xd = x.astype(np.float64)

    fdm = _CACHE["fdm"]
    in_maps = []
    for c in range(N_CORES):
        rsq = np.zeros((2, 2 * RES + 2 * E), np.float64)
        for n in range(2):
            cn = (masses[n] / 2.0) ** 2
            # lhsT: row0 = (x(a1)-px)^2/c, row1 = 1
            rsq[0, n * RES:(n + 1) * RES] = (xd - pos[n, 0]) ** 2 / cn
            rsq[1, n * RES:(n + 1) * RES] = 1.0
            # rhs: row0 = 1, row1 = ((y(a0)-py)^2 + (z-pz)^2)/c
            b = np.arange(EXTB)
            a0 = np.clip(c * PLANES + b - 1, 0, RES - 1)
            yterm = (xd[a0] - pos[n, 1]) ** 2
            zterm = (xd - pos[n, 2]) ** 2
            val = (yterm[:, None] + zterm[None, :]).reshape(-1) / cn
            rsq[0, 2 * RES + n * E:2 * RES + (n + 1) * E] = 1.0
            rsq[1, 2 * RES + n * E:2 * RES + (n + 1) * E] = val
        kb = np.full((RES, 2), 1.0, np.float32)
        if c == 0:
            kb[:, 0] = 2.0
        if c == N_CORES - 1:
            kb[:, 1] = 2.0
        in_maps.append({
            "rsq": rsq.astype(ml_dtypes.bfloat16),
            "fdm": fdm,
            "kb": kb,
        })
    return in_maps


# Gamma^i_{jk} = delta_ij W_k + delta_ik W_j - delta_jk W_i:
# per slot s = 9i+3j+k a list of (field c, sign)
_SLOT_TERMS = []
for _i in range(3):
    for _j in range(3):
        for _k in range(3):
            t = []
            if _i == _j:
                t.append((_k, 1.0))
            if _i == _k:
                t.append((_j, 1.0))
            if _j == _k:
                t.append((_i, -1.0))
            _SLOT_TERMS.append(t)


def kernel(BH_positions, BH_masses_presoftplus):
    from concourse.bass_utils import run_bass_kernel_spmd

    nc, x, fdm = _get_setup()
    in_maps = _build_inmaps(BH_positions, BH_masses_presoftplus, x)
    res = run_bass_kernel_spmd(nc, in_maps, list(range(N_CORES)))

    # host gather: [a1, a0l, c, z] per core -> W[a0, a1, z, c] f32
    parts = np.stack([
        np.asarray(res.results[c]["out"]).reshape(RES, PLANES, 3, RES)
        for c in range(N_CORES)
    ])  # [core, a1, a0l, c, z]
    # exact removal of the bf16(1/C_INT) identity-scale rounding: the device
    # W's all carry a factor 1/(u*C_INT) with u = bf16(1/C_INT)
    import ml_dtypes
    u = float(np.float64(np.array(1.0 / C_INT, dtype=ml_dtypes.bfloat16)))
    W = (parts.astype(np.float32) * np.float32(u * C_INT)).transpose(
        0, 2, 1, 4, 3
    ).reshape(RES, RES, RES, 3)  # [a0, a1, z, c]
    out = np.zeros((RES, RES, RES, 27), np.float32)
    for s, terms in enumerate(_SLOT_TERMS):
        for cfld, sgn in terms:
            if sgn > 0:
                out[..., s] += W[..., cfld]
            else:
                out[..., s] -= W[..., cfld]
    return np.ascontiguousarray(out).reshape(RES, RES, RES, 3, 3, 3)
